# revision 20
# baseline (speedup 1.0000x reference)
"""ASSA attention (sparse squared-relu + dense softmax blend) on 8 TRN2 cores.

Strategy (per core, SPMD; 32 (b,h) pairs sharded 4 per core):
  - Scores computed TRANSPOSED: scoresT[s, l] = K @ Q^T via PE with
    lhsT = K^T chunk [64,128], rhs = Q^T [64, L] (both built by PE transpose).
  - ACT computes E = exp(scoresT/8) (fp32r) straight from PSUM.
  - relu^2 path: relu (ACT or DVE) -> fp16, square on DVE (2x mode).
  - Two PV matmuls per s-chunk, accumulated in PSUM over all 16 s-chunks:
      ssaT  = (alpha1*V)^T @ R      [64, L]   (fp16 operands)
      dsaT  = [alpha2*V | 1]^T @ E  [65, L]   (fp32r) -- row 64 = softmax
                                               denominator for free.
  - Small output-side fixup: transpose [64/65, 128] tiles back with PE,
    out = ssa_t + dsa_t * (1/sum), DMA out.
All matmuls use float32r (1 cycle/row at N=512, ~1.5e-4 precision).
"""

import os
import sys

sys.path.insert(0, "/opt/trn_rl_repo")

import numpy as np

import concourse.bass as bass  # noqa: F401  (bass types used via tile/bacc)
import concourse.tile as tile
from concourse import bacc, mybir
from concourse.bass_utils import run_bass_kernel_spmd
from concourse.masks import make_identity

F32 = mybir.dt.float32
F32R = mybir.dt.float32r
FP16 = mybir.dt.float16
AF = mybir.ActivationFunctionType
ALU = mybir.AluOpType

N_CORES = 8
E = 64  # head dim
RELU_ACT_MOD = int(os.environ.get("RELU_ACT_MOD", "4"))  # 1/4 of relus on ACT
SQ_GPS_MOD = int(os.environ.get("SQ_GPS_MOD", "1000000"))  # squares stay on DVE


def build_kernel(nbh, L, S, alpha1, alpha2, n_devices=N_CORES):
    """Build the per-core SPMD program. Returns a compiled Bacc."""
    assert L % 1024 == 0 or L in (256, 512)
    LB = int(os.environ.get("KLB", "0")) or min(L, 512)  # L-block size
    n_lb = L // LB
    SC = S // 128              # number of s-chunks
    NP = LB // 512 if LB >= 512 else 1   # 512-wide matmul pieces per block
    PW = min(LB, 512)          # matmul piece width
    n_t = LB // 128            # output L-tiles per block

    nc = bacc.Bacc("TRN2", target_bir_lowering=False, debug=False,
                   num_devices=n_devices)
    q_d = nc.dram_tensor("q", [nbh, L, E], F32, kind="ExternalInput").ap()
    k_d = nc.dram_tensor("k", [nbh, S, E], F32, kind="ExternalInput").ap()
    v_d = nc.dram_tensor("v", [nbh, S, E], F32, kind="ExternalInput").ap()
    o_d = nc.dram_tensor("o", [nbh, L, E], F32, kind="ExternalOutput").ap()

    qk_scale = 1.0 / np.sqrt(E)

    with tile.TileContext(nc) as tc:
        with (
            tc.tile_pool(name="const", bufs=1) as constp,
            tc.tile_pool(name="inp", bufs=2) as inp,
            tc.tile_pool(name="wt", bufs=2) as wt,
            tc.tile_pool(name="ew", bufs=int(os.environ.get("EW_BUFS","3"))) as ew,
            tc.tile_pool(name="rw", bufs=int(os.environ.get("RW_BUFS","3"))) as rw,
            tc.tile_pool(name="osb", bufs=2) as osb,
            tc.tile_pool(name="tiny", bufs=4) as tiny,
            tc.tile_pool(name="sc", bufs=(4 if LB <= 512 else 2),
                         space="PSUM") as scp,
            tc.tile_pool(name="acc", bufs=1, space="PSUM") as accp,
            tc.tile_pool(name="blp", bufs=2, space="PSUM") as blp,
        ):
            ident = constp.tile([128, 128], F32, tag="ident")
            make_identity(nc, ident)

            LT = L // 128  # l-tiles
            ST = S // 128  # s-tiles

            def prep_steps(bh):
                """Emit-later thunks that load/transform inputs for `bh`.
                Returns (steps, handles); handles filled as steps run."""
                h = {}

                def dma_in():
                    h["q_in"] = inp.tile([128, LT, E], F32, tag="qin", name="q_in")
                    h["k_in"] = inp.tile([128, ST, E], F32, tag="kin", name="k_in")
                    h["v_in"] = inp.tile([128, ST, E], F32, tag="vin", name="v_in")
                    nc.sync.dma_start(
                        out=h["q_in"],
                        in_=q_d[bh].rearrange("(i p) e -> p i e", p=128))
                    nc.sync.dma_start(
                        out=h["k_in"],
                        in_=k_d[bh].rearrange("(i p) e -> p i e", p=128))
                    nc.sync.dma_start(
                        out=h["v_in"],
                        in_=v_d[bh].rearrange("(i p) e -> p i e", p=128))

                def v_prep():
                    # dsa stationary: [alpha2 * V | 1] (fp32r); ssa stationary:
                    # alpha1/64 * V (fp16) -- relu^2 path carries raw scores,
                    # so qk_scale^2 * alpha1 folds in here.
                    h["v_aug"] = wt.tile([128, ST, E + 1], F32R, tag="vaug", name="v_aug")
                    nc.gpsimd.tensor_scalar(
                        out=h["v_aug"][:, :, E:E + 1], in0=h["v_in"][:, :, 0:1],
                        scalar1=0.0, scalar2=1.0, op0=ALU.mult, op1=ALU.add)
                    nc.gpsimd.tensor_scalar(
                        out=h["v_aug"][:, :, 0:E], in0=h["v_in"],
                        scalar1=float(alpha2), scalar2=None, op0=ALU.mult)
                    h["v_bf"] = wt.tile([128, ST, E], FP16, tag="vbf", name="v_bf")
                    nc.gpsimd.tensor_scalar(
                        out=h["v_bf"], in0=h["v_in"],
                        scalar1=float(alpha1 / E), scalar2=None, op0=ALU.mult)

                def alloc_t():
                    h["qt"] = wt.tile([128, L], F32R, tag="qt", name="qt")
                    h["kt"] = wt.tile([128, S // 2], F32R, tag="kt", name="kt")

                def tr_group_q(g):
                    # 4 q l-tiles -> qt2 top rows [0:64]
                    gw = min(4, LT - g)
                    trp = blp if LB <= 512 else scp
                    tr = trp.tile([64, 512], F32,
                                  tag=("blp" if LB <= 512 else "sc"))
                    for i in range(gw):
                        nc.tensor.transpose(
                            tr[:, i * 128:(i + 1) * 128],
                            h["q_in"][:, g + i, :], ident)
                    nc.vector.tensor_copy(
                        h["qt"][0:64, g * 128:(g + gw) * 128], tr[:, 0:gw * 128])

                def dup_q():
                    # duplicate q^T into partitions 64..127 (free on engines)
                    nc.sync.dma_start(out=h["qt"][64:128, :],
                                      in_=h["qt"][0:64, :])

                def tr_group_k(g):
                    # 4 s-block PAIRS -> kt2 [128, 4*128]; pair 2j/2j+1
                    # lands on partitions 0:64 / 64:128 of column block j
                    gw = min(4, ST // 2 - g)
                    trp = blp if LB <= 512 else scp
                    tr = trp.tile([128, 512], F32,
                                  tag=("blp" if LB <= 512 else "sc"))
                    for i in range(gw):
                        pair = h["k_in"][:, 2 * (g + i):2 * (g + i) + 2, :]
                        nc.tensor.transpose(
                            tr[:, i * 128:(i + 1) * 128],
                            pair.rearrange("p c e -> p (c e)"), ident)
                    nc.scalar.activation(
                        h["kt"][:, g * 128:(g + gw) * 128], tr[:, 0:gw * 128],
                        AF.Copy)

                steps = [dma_in, v_prep, alloc_t]
                for g in range(0, LT, 4):
                    steps.append(lambda g=g: tr_group_q(g))
                steps.append(dup_q)
                for g in range(0, ST // 2, 4):
                    steps.append(lambda g=g: tr_group_k(g))
                return steps, h

            chunk_idx = 0  # global chunk counter for ACT/DVE relu balancing
            steps0, h0 = prep_steps(0)
            for st in steps0:
                st()
            cur = h0

            for bh in range(nbh):
                nxt_steps, nxt_h = prep_steps(bh + 1) if bh + 1 < nbh else ([], None)
                qt, kt = cur["qt"], cur["kt"]
                v_aug, v_bf = cur["v_aug"], cur["v_bf"]

                # ---- main loops ----
                for lb in range(n_lb):
                    ssa_ps = accp.tile([64, LB], F32, tag="accs")
                    dsa_ps = accp.tile([E + 1, LB], F32, tag="accd")
                    sc_pair = [None, None]
                    for s in range(SC):
                        # row-packed QK: s-blocks 2j (PE rows 0-63) and 2j+1
                        # (rows 64-127) compute concurrently
                        if s % 2 == 0:
                            j = s // 2
                            sc_pair[0] = scp.tile([128, LB], F32, tag="sc", name="sc_a")
                            sc_pair[1] = scp.tile([128, LB], F32, tag="sc", name="sc_b")
                            for c in range(NP):
                                cl = slice(c * PW, (c + 1) * PW)
                                ql = slice(lb * LB + c * PW,
                                           lb * LB + (c + 1) * PW)
                                nc.tensor.matmul(
                                    sc_pair[0][:, cl], kt[0:64, j * 128:(j + 1) * 128],
                                    qt[0:64, ql], start=True, stop=True,
                                    tile_position=(0, 0))
                                nc.tensor.matmul(
                                    sc_pair[1][:, cl], kt[64:128, j * 128:(j + 1) * 128],
                                    qt[64:128, ql], start=True, stop=True,
                                    tile_position=(64, 0))
                        sc_t = sc_pair[s % 2]
                        # exp path (fp32r)
                        e_t = ew.tile([128, LB], F32R, tag="e")
                        nc.scalar.activation(e_t, sc_t, AF.Exp, scale=qk_scale)
                        # relu path: relu(s) -> fp16 (scale folded into v_bf),
                        # then square. Engines alternate for load balance.
                        rl = rw.tile([128, LB], FP16, tag="rl")
                        if chunk_idx % RELU_ACT_MOD == 0:
                            nc.scalar.activation(rl, sc_t, AF.Relu)
                        else:
                            nc.vector.tensor_scalar(
                                out=rl, in0=sc_t, scalar1=0.0, scalar2=None,
                                op0=ALU.max)
                        r2 = rw.tile([128, LB], FP16, tag="r2")
                        if chunk_idx % SQ_GPS_MOD == 0:
                            nc.gpsimd.tensor_mul(r2, rl, rl)
                        else:
                            nc.vector.tensor_mul(r2, rl, rl)
                        chunk_idx += 1
                        # PV accumulation
                        first = s == 0
                        last = s == SC - 1
                        for c in range(NP):
                            sl = slice(c * PW, (c + 1) * PW)
                            nc.tensor.matmul(
                                ssa_ps[:, sl], v_bf[:, s, :], r2[:, sl],
                                start=first, stop=last)
                            nc.tensor.matmul(
                                dsa_ps[:, sl], v_aug[:, s, :], e_t[:, sl],
                                start=first, stop=last)
                        # interleave next-bh prep into this bh's chunk
                        # stream: input DMA at mid-bh, transforms during the
                        # last L-block (one small step per chunk)
                        bh_chunk = lb * SC + s
                        if nxt_steps and (
                                bh_chunk == (n_lb - 1) * SC // 2
                                or lb == n_lb - 1):
                            nxt_steps.pop(0)()
                    # ---- drain accumulators, transpose back, blend ----
                    ssa_sb = osb.tile([64, LB], F32, tag="ssasb")
                    dsa_sb = osb.tile([E + 1, LB], F32, tag="dsasb")
                    nc.vector.tensor_copy(ssa_sb, ssa_ps)
                    nc.scalar.activation(dsa_sb, dsa_ps, AF.Copy)
                    out_sb = osb.tile([128, n_t, E], F32, tag="outsb")
                    for t in range(n_t):
                        tsl = slice(t * 128, (t + 1) * 128)
                        trp = blp if LB <= 512 else scp
                        tr = trp.tile([128, 2 * E + 1], F32,
                                      tag=("blp" if LB <= 512 else "sc"))
                        nc.tensor.transpose(
                            tr[:, 0:E], ssa_sb[:, tsl], ident[0:64, 0:64])
                        nc.tensor.transpose(
                            tr[:, E:2 * E + 1], dsa_sb[:, tsl],
                            ident[0:E + 1, 0:E + 1])
                        rcp = tiny.tile([128, 1], F32, tag="rcp")
                        nc.vector.reciprocal(rcp, tr[:, 2 * E:2 * E + 1])
                        tmp = tiny.tile([128, E], F32, tag="tmp")
                        nc.scalar.activation(tmp, tr[:, E:2 * E], AF.Copy,
                                             scale=rcp)
                        nc.vector.tensor_add(out_sb[:, t, :], tmp, tr[:, 0:E])
                    nc.sync.dma_start(
                        out=o_d[bh, lb * LB:(lb + 1) * LB, :].rearrange(
                            "(t p) e -> p t e", p=128),
                        in_=out_sb)
                # flush any remaining prep for the next bh
                for st in nxt_steps:
                    st()
                cur = nxt_h

    nc.compile()
    return nc


def execute(inputs, **run_kwargs):
    """Run the full problem; returns (output, BassKernelResults)."""
    queries = np.asarray(inputs["queries"], dtype=np.float32)
    keys = np.asarray(inputs["keys"], dtype=np.float32)
    values = np.asarray(inputs["values"], dtype=np.float32)
    a1 = float(np.asarray(inputs["a1"]))
    a2 = float(np.asarray(inputs["a2"]))

    B, L, H, Edim = queries.shape
    assert Edim == E
    w1, w2 = np.exp(a1), np.exp(a2)
    alpha1 = w1 / (w1 + w2)
    alpha2 = w2 / (w1 + w2)

    # [B, L, H, E] -> [B*H, L, E]
    qh = np.ascontiguousarray(queries.transpose(0, 2, 1, 3)).reshape(B * H, L, E)
    kh = np.ascontiguousarray(keys.transpose(0, 2, 1, 3)).reshape(B * H, L, E)
    vh = np.ascontiguousarray(values.transpose(0, 2, 1, 3)).reshape(B * H, L, E)

    nbh = (B * H) // N_CORES
    nc = build_kernel(nbh, L, L, alpha1, alpha2)

    in_maps = []
    for i in range(N_CORES):
        sl = slice(i * nbh, (i + 1) * nbh)
        in_maps.append({"q": qh[sl], "k": kh[sl], "v": vh[sl]})

    res = run_bass_kernel_spmd(nc, in_maps, core_ids=list(range(N_CORES)),
                               **run_kwargs)
    out = np.concatenate([r["o"] for r in res.results], axis=0)  # [B*H, L, E]
    out = out.reshape(B, H, L, E).transpose(0, 2, 1, 3)
    return np.ascontiguousarray(out), res


def kernel(**inputs):
    out, _ = execute(inputs)
    return out


if __name__ == "__main__":
    # tiny smoke test: single core, small shapes
    rng = np.random.default_rng(0)
    nbh, L = 1, 256
    q = rng.standard_normal((nbh, L, E), dtype=np.float32)
    k = rng.standard_normal((nbh, L, E), dtype=np.float32)
    v = rng.standard_normal((nbh, L, E), dtype=np.float32)
    a1 = a2 = 1.0
    nc = build_kernel(nbh, L, L, 0.5, 0.5, n_devices=1)
    res = run_bass_kernel_spmd(
        nc, [{"q": q, "k": k, "v": v}], core_ids=[0]).results[0]
    got = res["o"].astype(np.float64)

    # numpy reference
    s = np.einsum("ble,bse->bls", q, k).astype(np.float64) / np.sqrt(E)
    ssa = np.maximum(s, 0) ** 2
    dsa = np.exp(s - s.max(-1, keepdims=True))
    dsa /= dsa.sum(-1, keepdims=True)
    ref = 0.5 * np.einsum("bls,bse->ble", ssa, v) + \
        0.5 * np.einsum("bls,bse->ble", dsa, v)
    print("l2_rel:", np.linalg.norm(got - ref) / np.linalg.norm(ref))
